# revision 68
# baseline (speedup 1.0000x reference)
"""Multi-head cross-attention Trainium2 kernel (bf16 pipeline).

Full-input contract: kernel(**inputs) takes the complete tensors and returns
the complete output. Internally shards over 8 NeuronCores as
(batch x head-group): core c handles batch c//4 and heads [4*(c%4), 4*(c%4)+4).
Each core computes its partial output  ctx_g @ Wo_g  for its batch; the host
sums the 4 head-group partials per batch and adds bo_eff = bo + bv @ Wo
(the V bias commutes through attention, so it is applied on the host).

Masked keys (key_mask == 0) contribute exactly zero probability, so the host
compacts key/value to the unmasked rows (padded up to a multiple of 128 with
-1e9 score bias), which shrinks the K/V projections and the whole attention
core proportionally.

All activations/weights are bf16 on device (fp32 PSUM accumulation), which
runs the PE at full rate and halves DMA/SBUF traffic. Per-core pipeline:
  qT = (Wq_g^T @ query_b^T)            [256, 1024]  (dh on partitions)
  kT = (Wk_g^T @ key_b^T)              [256, SKP]
  v  = (value_b @ Wv_g)                [SKP, 256]   (sk on partitions)
  per (head h, sk-tile skt): S^T[sk,128..] = kT_h x qT_h; es = exp via ACT
  with fused scale + per-partition mask bias; ctx[sq,d] and denominators
  accumulate in PSUM over skt with es tiles as the stationary operand
  (lhsT), so ctx needs no giant transposed accumulators; normalize with the
  reciprocal denominator on the PSUM->SBUF move; transpose ctx tiles on the
  PE; out = ctxT^T @ Wo_g.
K/V projection of sk-block j+1 is interleaved into the attention over block
j so the PE stays busy while ACT computes the softmax exps.
"""

import numpy as np
import ml_dtypes

B, SQ, SK, IN = 2, 1024, 4096, 1024
H_TOT, D, HPC = 16, 64, 4
DH = HPC * D  # 256, per-core head-dim slice
NCORES = 8
BF_NP = ml_dtypes.bfloat16

_CACHE = {}

# scheduling knobs
CFG = {"pp": 2, "ses": 8, "sin": 5, "sout": 4, "warm": 16, "delay": 5}


def _blocks_of(width, step=512):
    out, off = [], 0
    while off < width:
        w = min(step, width - off)
        out.append((off, w))
        off += w
    return out


def _build(skp):
    import concourse.tile as tile
    from concourse import bacc, mybir

    FP = mybir.dt.float32
    BF = mybir.dt.bfloat16
    AF = mybir.ActivationFunctionType

    nc = bacc.Bacc("TRN2", target_bir_lowering=False, debug=False)

    qT_d = nc.dram_tensor("qT", [IN, SQ], BF, kind="ExternalInput").ap()
    kT_d = nc.dram_tensor("kT", [IN, skp], BF, kind="ExternalInput").ap()
    vT_d = nc.dram_tensor("vT", [IN, skp], BF, kind="ExternalInput").ap()
    wq_d = nc.dram_tensor("wq", [IN, DH], BF, kind="ExternalInput").ap()
    wk_d = nc.dram_tensor("wk", [IN, DH], BF, kind="ExternalInput").ap()
    wv_d = nc.dram_tensor("wv", [IN, DH], BF, kind="ExternalInput").ap()
    wo_d = nc.dram_tensor("wo", [DH, SQ], BF, kind="ExternalInput").ap()
    bqk_d = nc.dram_tensor("bqk", [128, 4], FP, kind="ExternalInput").ap()
    mb_d = nc.dram_tensor("mb", [128, skp // 128], FP, kind="ExternalInput").ap()
    ones_d = nc.dram_tensor("ones", [128, 1], BF, kind="ExternalInput").ap()
    idn_d = nc.dram_tensor("idn", [128, 128], BF, kind="ExternalInput").ap()
    out_d = nc.dram_tensor("out", [SQ, SQ], FP, kind="ExternalOutput").ap()

    NSKT = skp // 128          # sk tiles of 128
    NKC = IN // 128            # 8 contraction chunks
    SCALE = 1.0 / float(np.sqrt(D))
    blocks = _blocks_of(skp)

    with tile.TileContext(nc) as tc:
        cpool_cm = tc.tile_pool(name="const", bufs=1)
        cpool = cpool_cm.__enter__()
        wq_sb = cpool.tile([128, NKC, DH], BF, name="wq_sb")
        wk_sb = cpool.tile([128, NKC, DH], BF, name="wk_sb")
        wv_sb = cpool.tile([128, NKC, DH], BF, name="wv_sb")
        wo_sb = cpool.tile([128, 2, SQ], BF, name="wo_sb")
        bqk_sb = cpool.tile([128, 4], FP, name="bqk_sb")
        mb_sb = cpool.tile([128, NSKT], FP, name="mb_sb")
        ones_sb = cpool.tile([128, 1], BF, name="ones_sb")
        idn_sb = cpool.tile([128, 128], BF, name="idn_sb")
        qT_sb = cpool.tile([128, 2, SQ], BF, name="qT_sb")
        kT_sb = cpool.tile([128, 2, skp], BF, name="kT_sb")
        v_sb = cpool.tile([128, NSKT, DH], BF, name="v_sb")
        junk_sb = cpool.tile([64, 512], BF, name="junk_sb")
        ctx_sbs = [cpool.tile([128, DH], BF, name=f"ctx_sb{s}")
                   for s in range(SQ // 128)]
        ctxT_sbs = [cpool.tile([128, 2, 128], BF, name=f"ctxT_sb{s}")
                    for s in range(SQ // 128)]
        rec_sb = cpool.tile([128, 32], FP, name="rec_sb")

        def dma(dst, src):
            nc.sync.dma_start(out=dst, in_=src)

        # critical-path loads first: every DMA costs a ~650ns SP issue
        # slot, so even tiny constant loads are deferred until after the
        # first kT block is in flight.
        dma(wq_sb[:], wq_d.rearrange("(kc p) n -> p kc n", p=128))
        dma(bqk_sb[:], bqk_d[:, :])
        dma(mb_sb[:], mb_d[:, :])
        nc.gpsimd.memset(junk_sb[:], 0.0)

        with tc.tile_pool(name="sin", bufs=CFG["sin"]) as sin, \
             tc.tile_pool(name="ses", bufs=CFG["ses"]) as ses, \
             tc.tile_pool(name="sout", bufs=CFG["sout"]) as sout, \
             tc.tile_pool(name="pp", bufs=CFG["pp"], space="PSUM") as pp, \
             tc.tile_pool(name="pj", bufs=1, space="PSUM") as pj, \
             tc.tile_pool(name="pa", bufs=1, space="PSUM") as pa, \
             tc.tile_pool(name="pd", bufs=1, space="PSUM") as pd:

            pa_t = pa.tile([128, HPC * (SQ // 128) * 64], FP, tag="pa", name="pa_t")
            pd_t = pd.tile([128, HPC * (SQ // 128)], FP, tag="pd", name="pd_t")

            def load_block(x_d, off, w, name):
                xin = sin.tile([128, NKC, 512], BF, tag="sin", name=name)
                dma(xin[:, :, 0:w],
                    x_d.rearrange("(kc p) n -> p kc n", p=128)[:, :, off:off + w])
                return xin

            def qk_proj_a(w_sb, xin, off, w, t, src=0):
                ps = pp.tile([128, 512], FP, tag="pp", name="ps")
                for kc in range(NKC // 2):
                    nc.tensor.matmul(
                        ps[:, 0:w],
                        lhsT=w_sb[:, kc, t * 128:(t + 1) * 128],
                        rhs=xin[:, kc, src:src + w],
                        start=(kc == 0), stop=False)
                return ps

            def qk_proj_b(ps, w_sb, xin, dst_sb, bias_col0, off, w, t, src=0):
                for kc in range(NKC // 2, NKC):
                    nc.tensor.matmul(
                        ps[:, 0:w],
                        lhsT=w_sb[:, kc, t * 128:(t + 1) * 128],
                        rhs=xin[:, kc, src:src + w],
                        start=False, stop=(kc == NKC - 1))
                with nc.allow_low_precision(reason="bf16 storage"):
                    nc.vector.tensor_scalar_add(
                        dst_sb[:, t, off:off + w], ps[:, 0:w],
                        bqk_sb[:, bias_col0 + t:bias_col0 + t + 1])

            def qk_proj(w_sb, xin, dst_sb, bias_col0, off, w, t, src=0):
                ps = qk_proj_a(w_sb, xin, off, w, t, src)
                qk_proj_b(ps, w_sb, xin, dst_sb, bias_col0, off, w, t, src)

            def qk_part(w_sb, xin, dst_sb, bias_col0, off, w, t, part, key,
                        src=0):
                # quarter-sized projection work item (2 of 8 kc chunks)
                if part == 0:
                    state[key] = pj.tile([128, 512], FP, tag="pj", name="ps")
                ps = state[key]
                for kc in (2 * part, 2 * part + 1):
                    nc.tensor.matmul(
                        ps[:, 0:w],
                        lhsT=w_sb[:, kc, t * 128:(t + 1) * 128],
                        rhs=xin[:, kc, src:src + w],
                        start=(kc == 0), stop=(kc == NKC - 1))
                if part == 3:
                    with nc.allow_low_precision(reason="bf16 storage"):
                        nc.vector.tensor_scalar_add(
                            dst_sb[:, t, off:off + w], ps[:, 0:w],
                            bqk_sb[:, bias_col0 + t:bias_col0 + t + 1])

            def v_part(xin, off, j, part, key):
                if part == 0:
                    state[key] = pj.tile([128, DH], FP, tag="pj", name="psv")
                ps = state[key]
                for kc in range(4 * part, 4 * part + 4):
                    nc.tensor.matmul(
                        ps[:, :],
                        lhsT=xin[:, kc, j * 128:(j + 1) * 128],
                        rhs=wv_sb[:, kc, :],
                        start=(kc == 0), stop=(kc == NKC - 1))
                if part == 1:
                    with nc.allow_low_precision(reason="bf16 storage"):
                        nc.vector.tensor_copy(
                            v_sb[:, off // 128 + j, :], ps[:, :])

            def v_proj(xin, off, j):
                # one sk tile of 128: v[sk, dh] = value_chunk^T-stationary matmul
                skt = off // 128 + j
                ps = pp.tile([128, DH], FP, tag="pp", name="psv")
                for kc in range(NKC):
                    nc.tensor.matmul(
                        ps[:, :],
                        lhsT=xin[:, kc, j * 128:(j + 1) * 128],
                        rhs=wv_sb[:, kc, :],
                        start=(kc == 0), stop=(kc == NKC - 1))
                with nc.allow_low_precision(reason="bf16 storage"):
                    nc.vector.tensor_copy(v_sb[:, skt, :], ps[:, :])

            def scores_exp(h, skt, half):
                t, r0 = h // 2, 64 * (h % 2)
                ps_s = pp.tile([128, 512], FP, tag="pp", name="ps_s")
                nc.tensor.matmul(
                    ps_s[:, :],
                    lhsT=kT_sb[r0:r0 + 64, t, skt * 128:(skt + 1) * 128],
                    rhs=qT_sb[r0:r0 + 64, t, half * 512:(half + 1) * 512],
                    start=True, stop=True)
                es = ses.tile([128, 512], BF, tag="es", name="es")
                with nc.allow_low_precision(reason="bf16 storage"):
                    nc.scalar.activation(
                        es[:, :], ps_s[:, :], AF.Exp,
                        bias=mb_sb[:, skt:skt + 1], scale=SCALE)
                return es

            def ctx_acc(es, h, skt, half):
                # psum accumulation groups are per 2KB zero region (bank):
                # only the first matmul into a bank starts it, only the very
                # last stops it. pa bank = head h; pd is one shared bank.
                for sqt in range(4):
                    g = half * 4 + sqt
                    nc.tensor.matmul(
                        pa_t[:, (h * 8 + g) * 64:(h * 8 + g) * 64 + 64],
                        lhsT=es[:, sqt * 128:(sqt + 1) * 128],
                        rhs=v_sb[:, skt, h * 64:(h + 1) * 64],
                        start=(skt == 0 and g == 0),
                        stop=(skt == NSKT - 1 and g == 7))
                    nc.tensor.matmul(
                        pd_t[:, h * 8 + g:h * 8 + g + 1],
                        lhsT=es[:, sqt * 128:(sqt + 1) * 128],
                        rhs=ones_sb[:, 0:1],
                        start=(skt == 0 and h == 0 and g == 0),
                        stop=(skt == NSKT - 1 and h == HPC - 1 and g == 7))

            # ---- prologue: just enough for skt 0 of head 0 to start ----
            # qT half 0, kT/v for sk tiles 0-1. Everything else rides the
            # paced work queue below. PE warms its p-state on junk matmuls
            # while the first DMAs are in flight.
            w0 = blocks[0][1]
            wp0 = min(256, w0)
            qin0 = sin.tile([128, NKC, 512], BF, tag="sin", name="qin0")
            dma(qin0[:, :, 0:256],
                qT_d.rearrange("(kc p) n -> p kc n", p=128)[:, :, 0:256])
            dma(qin0[:, :, 256:512],
                qT_d.rearrange("(kc p) n -> p kc n", p=128)[:, :, 256:512])
            dma(wk_sb[:], wk_d.rearrange("(kc p) n -> p kc n", p=128))
            kin0 = sin.tile([128, NKC, 512], BF, tag="sin", name="kin0")
            dma(kin0[:, :, 0:wp0],
                kT_d.rearrange("(kc p) n -> p kc n", p=128)[:, :, 0:wp0])
            dma(wv_sb[:], wv_d.rearrange("(kc p) n -> p kc n", p=128))
            vin0 = sin.tile([128, NKC, 512], BF, tag="sin", name="vin0")
            dma(vin0[:, :, 0:wp0],
                vT_d.rearrange("(kc p) n -> p kc n", p=128)[:, :, 0:wp0])
            dma(ones_sb[:], ones_d[:, :])
            qin1 = load_block(qT_d, 512, 512, "qin1")
            if w0 > wp0:
                dma(kin0[:, :, wp0:w0],
                    kT_d.rearrange("(kc p) n -> p kc n", p=128)[:, :, wp0:w0])
                dma(vin0[:, :, wp0:w0],
                    vT_d.rearrange("(kc p) n -> p kc n", p=128)[:, :, wp0:w0])
            dma(idn_sb[:], idn_d[:, :])
            dma(wo_sb[:], wo_d.rearrange("(t p) n -> p t n", p=128))

            for i in range(CFG.get("warm", 17)):
                wm = pj.tile([64, 512], FP, tag="pj", name="wm")
                nc.tensor.matmul(wm[:, :], lhsT=junk_sb[:, 0:64],
                                 rhs=junk_sb[:, :], start=True, stop=True)

            qk_proj(wq_sb, qin0, qT_sb, 0, 0, 256, 0)
            qk_proj(wq_sb, qin0, qT_sb, 0, 256, 256, 0, src=256)
            qk_proj(wk_sb, kin0, kT_sb, 2, 0, wp0, 0)

            # ---- interleaved proj work queue for blocks 1.. ----
            # items of ~4 matmuls each, paced by step index (8 steps per
            # skt) so ACT never starves behind a long projection burst.
            # DMAs are issued ~2 skt before their matmuls run.
            work = []
            xins = {}
            state = {}
            # block-0 remainder + Q half-1 projection, paced through the
            # first two skts (whose t0-half0 steps run first; see step
            # order below). All fine-grained parts on the pj pool.
            def qkw(due, w_sb, xin, dst, b0, off, w, t, key, src=0):
                for part in range(4):
                    work.append((due[part],
                                 lambda part=part: qk_part(
                                     w_sb, xin, dst, b0, off, w, t, part,
                                     key, src)))
            def vw(due, xin, off, j, key):
                for part in range(2):
                    work.append((due[part],
                                 lambda part=part: v_part(
                                     xin, off, j, part, key)))
            # dues sit just before each item's first consumer: popping a
            # DMA-gated item early gives it a LOW priority, so when its
            # data lands it cuts ahead of already-ready scores matmuls
            # (priority inversion) and stalls ACT.
            qkw([0, 0, 1, 1], wq_sb, qin0, qT_sb, 0, 0, 512, 1, "qt1")
            qkw([2, 2, 3, 3], wk_sb, kin0, kT_sb, 2, 0, wp0, 1, "kt1")
            vw([3, 4], vin0, 0, 0, "vj0")
            vw([5, 6], vin0, 0, 1, "vj1")
            qkw([6, 6, 7, 7], wq_sb, qin1, qT_sb, 0, 512, 512, 0, "qb1t0")
            qkw([8, 8, 9, 9], wq_sb, qin1, qT_sb, 0, 512, 512, 1, "qb1t1")
            if w0 > wp0:
                qkw([10, 10, 11, 11], wk_sb, kin0, kT_sb, 2, wp0, w0 - wp0,
                    0, "k0r0", src=wp0)
                qkw([12, 12, 13, 13], wk_sb, kin0, kT_sb, 2, wp0, w0 - wp0,
                    1, "k0r1", src=wp0)
            for j in range(2, w0 // 128):
                vw([10 + 2 * j, 11 + 2 * j], vin0, 0, j, f"vj{j}")
            for bi in range(1, len(blocks)):
                off, w = blocks[bi]
                # deadlines: S(skt 4*bi) is emitted at step 32*bi, so the
                # kT projection parts must pop before that; v_proj for skt
                # s is consumed by ctx at step 8*s + DELAY.
                def mk_dma(off=off, w=w, bi=bi):
                    xins[("k", bi)] = load_block(kT_d, off, w, f"kin{bi}")
                    xins[("v", bi)] = load_block(vT_d, off, w, f"vin{bi}")
                work.append((max(0, 32 * bi - 22) if bi > 1 else 4, mk_dma))
                for t in range(2):
                    for part in range(4):
                        work.append((
                            32 * bi - 14 + 6 * t + part,
                            lambda t=t, part=part, off=off, w=w, bi=bi:
                            qk_part(wk_sb, xins[("k", bi)], kT_sb, 2,
                                    off, w, t, part, ("ps", bi, t))))
                for j in range(w // 128):
                    for part in range(2):
                        work.append((
                            32 * bi + 8 * j - 6 + 2 * part,
                            lambda off=off, bi=bi, j=j, part=part:
                            v_part(xins[("v", bi)], off, j, part,
                                   ("psv", bi, j))))
            work.sort(key=lambda x: x[0])      # stable: preserves dep order
            work.reverse()  # pop from end

            # ---- main attention loop ----
            # ctx matmuls consume es tiles DELAY steps behind the exp that
            # produced them, so their semaphores are already fired when the
            # PE sequencer reaches them (wait queue is only 4 deep).
            DELAY = CFG.get("delay", 2)
            # skt 0/1 run heads 0/1 (weight tile t0) first, then heads
            # 2/3 (t1), then the half-1 passes: the t1 and half-1
            # projections have late-landing DMAs and ride the work queue.
            steps = ([(0, 0, 0), (0, 1, 0), (1, 0, 0), (1, 1, 0),
                      (0, 2, 0), (0, 3, 0), (1, 2, 0), (1, 3, 0),
                      (0, 0, 1), (0, 1, 1), (1, 0, 1), (1, 1, 1),
                      (0, 2, 1), (0, 3, 1), (1, 2, 1), (1, 3, 1)]
                     + [(skt, h, half)
                        for skt in range(2, NSKT)
                        for half in (0, 1)
                        for h in range(HPC)])
            pending = []
            for i, (skt, h, half) in enumerate(steps):
                es = scores_exp(h, skt, half)
                pending.append((es, h, skt, half))
                if len(pending) > DELAY:
                    ctx_acc(*pending.pop(0))
                while work and work[-1][0] <= i:
                    work.pop()[1]()
            while work:
                work.pop()[1]()
            while pending:
                ctx_acc(*pending.pop(0))

            # ---- tail: normalize, transpose, output projection ----
            # per-sqt tiles + one merged loop so sq tiles pipeline across
            # engines. Work split DVE/ACT so neither paces the chain.
            nc.vector.reciprocal(rec_sb[:, :], pd_t[:, :])
            # all normalizes first: each engine's in-order queue would
            # otherwise serialize sqt N+1's normalize behind sqt N's
            # end-of-chain copy.
            for sqt in range(SQ // 128):
                ctx_s = ctx_sbs[sqt]
                for h in range(HPC):
                    c = h * 8 + sqt
                    with nc.allow_low_precision(reason="bf16 storage"):
                        if h < 2:
                            nc.vector.tensor_scalar_mul(
                                ctx_s[:, h * 64:(h + 1) * 64],
                                pa_t[:, c * 64:c * 64 + 64],
                                rec_sb[:, c:c + 1])
                        else:
                            nc.scalar.activation(
                                ctx_s[:, h * 64:(h + 1) * 64],
                                pa_t[:, c * 64:c * 64 + 64],
                                AF.Copy, scale=rec_sb[:, c:c + 1])
            for sqt in range(SQ // 128):
                ctx_s, ctxT_s = ctx_sbs[sqt], ctxT_sbs[sqt]
                for t in range(2):
                    # t0 reuses pd's bank (dead after the reciprocal); t1
                    # goes through pj so the two transposes pipeline.
                    pool = pd if t == 0 else pj
                    ps_t = pool.tile([128, 128], BF,
                                     tag="pd" if t == 0 else "pj",
                                     name="ps_t")
                    nc.tensor.transpose(
                        ps_t[:, :], in_=ctx_s[:, t * 128:(t + 1) * 128],
                        identity=idn_sb[:, :])
                    with nc.allow_low_precision(reason="bf16 storage"):
                        nc.vector.tensor_copy(ctxT_s[:, t, :], ps_t[:, :])
                o_sb = sout.tile([128, SQ], FP, tag="o", name="o_sb")
                for li, lo in enumerate((0, 512)):
                    ps_o = pp.tile([128, 512], FP, tag="pp", name="ps_o")
                    for t in range(2):
                        nc.tensor.matmul(
                            ps_o[:, :],
                            lhsT=ctxT_s[:, t, :],
                            rhs=wo_sb[:, t, lo:lo + 512],
                            start=(t == 0), stop=(t == 1))
                    if li == 0:
                        nc.vector.tensor_copy(o_sb[:, lo:lo + 512], ps_o[:, :])
                    else:
                        nc.scalar.copy(o_sb[:, lo:lo + 512], ps_o[:, :])
                    dma(out_d[sqt * 128:(sqt + 1) * 128, lo:lo + 512],
                        o_sb[:, lo:lo + 512])

        cpool_cm.__exit__(None, None, None)

    nc.compile()
    return nc


def get_nc(skp=SK):
    key = ("nc", skp)
    if key not in _CACHE:
        _CACHE[key] = _build(skp)
    return _CACHE[key]


def make_in_maps(query, key, value, key_mask, Wq, bq, Wk, bk, Wv, bv, Wo, bo):
    f32 = lambda x: np.asarray(x, dtype=np.float32)
    bf = lambda x: np.ascontiguousarray(np.asarray(x, dtype=np.float32),
                                        dtype=np.float32).astype(BF_NP)
    query, key, value = f32(query), f32(key), f32(value)
    Wq, bq, Wk, bk = f32(Wq), f32(bq), f32(Wk), f32(bk)
    Wv, Wo = f32(Wv), f32(Wo)
    key_mask = np.asarray(key_mask)

    # compact unmasked keys; pad to a common multiple of 128
    keep = [np.nonzero(key_mask[b] != 0)[0] for b in range(B)]
    skp = max(512, int(-(-max(len(k) for k in keep) // 128) * 128))
    skp = min(skp, SK)

    idn = np.eye(128, dtype=np.float32).astype(BF_NP)
    ones = np.ones((128, 1), np.float32).astype(BF_NP)
    qT, kT, vT, mb = [], [], [], []
    for b in range(B):
        n = len(keep[b])
        kc = np.zeros((skp, IN), np.float32)
        vc = np.zeros((skp, IN), np.float32)
        kc[:n] = key[b][keep[b]]
        vc[:n] = value[b][keep[b]]
        mbias = np.full(skp, -1e9, np.float32)
        mbias[:n] = 0.0
        qT.append(np.ascontiguousarray(query[b].T).astype(BF_NP))
        kT.append(np.ascontiguousarray(kc.T).astype(BF_NP))
        vT.append(np.ascontiguousarray(vc.T).astype(BF_NP))
        mb.append(np.ascontiguousarray(mbias.reshape(skp // 128, 128).T))

    in_maps = []
    for c in range(NCORES):
        b, g = c // 4, c % 4
        S = slice(DH * g, DH * (g + 1))
        bqk = np.stack([bq[S][0:128], bq[S][128:256],
                        bk[S][0:128], bk[S][128:256]], axis=1)
        in_maps.append({
            "qT": qT[b], "kT": kT[b], "vT": vT[b],
            "wq": np.ascontiguousarray(Wq[:, S]).astype(BF_NP),
            "wk": np.ascontiguousarray(Wk[:, S]).astype(BF_NP),
            "wv": np.ascontiguousarray(Wv[:, S]).astype(BF_NP),
            "wo": np.ascontiguousarray(Wo[S, :]).astype(BF_NP),
            "bqk": np.ascontiguousarray(bqk),
            "mb": mb[b], "ones": ones, "idn": idn,
        })
    return in_maps, skp


def run(in_maps, skp=SK, trace=False):
    from concourse.bass_utils import run_bass_kernel_spmd
    nc = get_nc(skp)
    res = run_bass_kernel_spmd(nc, in_maps, list(range(NCORES)), trace=trace)
    _CACHE["last_results"] = res
    return res


def kernel(query, key, value, key_mask, Wq, bq, Wk, bk, Wv, bv, Wo, bo):
    in_maps, skp = make_in_maps(query, key, value, key_mask,
                                Wq, bq, Wk, bk, Wv, bv, Wo, bo)
    res = run(in_maps, skp)
    out = np.zeros((B, SQ, SQ), np.float32)
    for c in range(NCORES):
        out[c // 4] += res.results[c]["out"]
    bo_eff = np.asarray(bo, np.float32) + f32v(bv) @ np.asarray(Wo, np.float32)
    out += bo_eff[None, None, :]
    return out


def f32v(x):
    return np.asarray(x, dtype=np.float32)
